# revision 9
# baseline (speedup 1.0000x reference)
"""Trainium2 Bass kernel for nn_BigramModel (8-layer dense transformer + LM head).

Self-contained: hardcodes all shapes from the problem spec.
Sharding: data-parallel over batch (16 rows -> 2 rows/core on 8 cores).
All matmuls run in float32r (TF32-like, full PE rate); activations are kept
in transposed [d, token] layout so weights stream in their native
[d_in, d_out] layout as the stationary operand.

kernel(**inputs) -> (logits [16,256,32000] f32, loss scalar f32)
"""
import sys
import types
import numpy as np
from contextlib import ExitStack

# NTFF profile hook shim (agent image's antenv lacks axon_hooks).
try:
    import antenv.axon_hooks  # noqa: F401
except ModuleNotFoundError:
    try:
        from trn_agent_boot.trn_boot import _ntff_profile_via_ctypes
        _m = types.ModuleType("antenv.axon_hooks")
        _hook = _ntff_profile_via_ctypes("/opt/axon/libaxon_pjrt.so")
        _m.get_axon_ntff_profile_hook = lambda: _hook
        sys.modules["antenv.axon_hooks"] = _m
    except Exception:
        pass

import concourse.bass as bass  # noqa: F401
import concourse.tile as tile
import concourse.bacc as bacc
import concourse.mybir as mybir
from concourse.bass_utils import run_bass_kernel_spmd

DT = mybir.dt
AF = mybir.ActivationFunctionType
ALU = mybir.AluOpType
AX = mybir.AxisListType

# model dims
V, T, D, H, L, HS = 32000, 256, 512, 8, 8, 64
B = 16
EPS = 1e-5
SCALE = float(D) ** -0.5
NCORES = 8
BL = B // NCORES            # batch rows per core (2)
NTOK = BL * T               # tokens per core (512)
KT = D // 128               # d-tiles (4)
VPAD = 32768                # padded vocab
VC = 1024                   # lm-head vocab chunk
NVC = VPAD // VC            # 32 chunks
F1 = 4 * D                  # ffn dim 2048
NEG = -1.0e30
N_PAD = VPAD - V            # 768 zero-padded vocab entries


def build_program(n_layers=L, n_vc=NVC, with_biases=frozenset(), ln_affine=True):
    """Emit the per-core SPMD program. with_biases: subset of
    {"proj","fc1","fc2","lm"} for which bias matmuls are emitted."""
    nc = bacc.Bacc("TRN2", target_bir_lowering=False, debug=False)

    def din(name, shape):
        return nc.dram_tensor(name, shape, DT.float32, kind="ExternalInput").ap()

    x0t = din("x0t", [D, NTOK])
    wkt = din("wkt", [L, D, D])
    wqt = din("wqt", [L, D, D])
    wvt = din("wvt", [L, D, D])
    wpj = din("wproj", [L, D, D])
    w1 = din("w1", [L, D, F1])
    w2 = din("w2", [L, F1, D])
    bpj = din("bproj", [L, D])
    b1d = din("b1", [L, F1])
    b2d = din("b2", [L, D])
    ln1s = din("ln1s", [L, D])
    ln1b = din("ln1b", [L, D])
    ln2s = din("ln2s", [L, D])
    ln2b = din("ln2b", [L, D])
    lnfs = din("lnfs", [1, D])
    lnfb = din("lnfb", [1, D])
    wlm = din("wlm", [D, VPAD])
    blm = din("blm", [1, VPAD])
    maskt = din("maskt", [T, T])
    onesd = din("ones", [128, 512])

    logits = nc.dram_tensor("logits", [NTOK, VPAD], DT.float32, kind="ExternalOutput").ap()
    ssum = nc.dram_tensor("ssum", [KT, 128], DT.float32, kind="ExternalOutput").ap()

    f32r = DT.float32r
    NZ = F1 // 128  # 16

    with tile.TileContext(nc) as tc, ExitStack() as ctx, \
            nc.allow_low_precision(reason="float32r matmul pipeline by design"):
        st = ctx.enter_context(tc.tile_pool(name="state", bufs=1))
        wp = ctx.enter_context(tc.tile_pool(name="wp", bufs=6))      # [128,512] f32r, tag wt
        w1p = ctx.enter_context(tc.tile_pool(name="w1p", bufs=6))    # [128,512] f32r, tag wt1
        w2p = ctx.enter_context(tc.tile_pool(name="w2p", bufs=4))    # [128,512] f32r, tag w2
        ztp = ctx.enter_context(tc.tile_pool(name="ztp", bufs=3))
        sqp = ctx.enter_context(tc.tile_pool(name="sqp", bufs=2))
        app = ctx.enter_context(tc.tile_pool(name="app", bufs=2))
        pp_sb = ctx.enter_context(tc.tile_pool(name="ppsb", bufs=2))
        tiny = ctx.enter_context(tc.tile_pool(name="tiny", bufs=2))
        rows = ctx.enter_context(tc.tile_pool(name="rows", bufs=2))
        wlmp = ctx.enter_context(tc.tile_pool(name="wlmp", bufs=6))  # [128,1024] f32r
        scrp = ctx.enter_context(tc.tile_pool(name="scrp", bufs=2))  # [128,1024] f32

        # ---- persistent state tiles ----
        x_sb = st.tile([128, KT, NTOK], f32r, tag="x")
        h_sb = st.tile([128, KT, NTOK], f32r, tag="h")
        kT_sb = st.tile([128, KT, NTOK], f32r, tag="kT")
        qT_sb = st.tile([128, KT, NTOK], f32r, tag="qT")
        v_sb = st.tile([128, KT, H, HS + 1], f32r, tag="v")
        at_sb = st.tile([128, KT, NTOK], f32r, tag="at")
        ones_sb = st.tile([128, 512], f32r, tag="ones")
        mask_sb = st.tile([128, 2, T], f32r, tag="mask")
        sacc_sb = st.tile([128, KT, n_vc], DT.float32, tag="sacc")
        sbuf_s = st.tile([128, KT], DT.float32, tag="sfin")
        eps_sb = st.tile([1, 1], DT.float32, tag="eps")
        nc.vector.memset(eps_sb[:], EPS)

        nc.sync.dma_start(ones_sb[:], onesd.bitcast(f32r))
        for j in range(2):
            nc.sync.dma_start(mask_sb[:, j, :], maskt[j * 128:(j + 1) * 128, :].bitcast(f32r))
        for k in range(KT):
            nc.sync.dma_start(x_sb[:, k, :], x0t[k * 128:(k + 1) * 128, :].bitcast(f32r))

        def emit_ln(pp, pp4, s_dram, b_dram, out_tile):
            """LayerNorm over d (partitions): x_sb -> out_tile."""
            if ln_affine:
                lnst = rows.tile([128, KT], DT.float32, tag="lnst")
                lnbt = rows.tile([128, KT], DT.float32, tag="lnbt")
                nc.sync.dma_start(lnst[:], s_dram.rearrange("(c p) -> p c", p=128))
                nc.sync.dma_start(lnbt[:], b_dram.rearrange("(c p) -> p c", p=128))

            s1 = pp4.tile([1, NTOK], DT.float32, tag="mix")
            s2 = pp4.tile([1, NTOK], DT.float32, tag="mix")
            for k in range(KT):
                sq = sqp.tile([128, NTOK], f32r, tag="sq")
                xk = x_sb[:, k, :]
                nc.vector.tensor_tensor(sq[:], xk, xk, ALU.mult)
                nc.tensor.matmul(s1[:], lhsT=ones_sb[:, 0:1], rhs=xk,
                                 start=(k == 0), stop=(k == KT - 1))
                nc.tensor.matmul(s2[:], lhsT=ones_sb[:, 0:1], rhs=sq[:],
                                 start=(k == 0), stop=(k == KT - 1))
            tm = tiny.tile([1, NTOK], f32r, tag="tm")
            tm2 = tiny.tile([1, NTOK], DT.float32, tag="lnt")
            var = tiny.tile([1, NTOK], DT.float32, tag="lnt")
            stdt = tiny.tile([1, NTOK], DT.float32, tag="lnt")
            rt = tiny.tile([1, NTOK], f32r, tag="rt")
            nc.vector.tensor_scalar_mul(tm[:], s1[:], 1.0 / D)
            nc.vector.tensor_tensor(tm2[:], tm[:], tm[:], ALU.mult)
            nc.vector.scalar_tensor_tensor(var[:], s2[:], 1.0 / D, tm2[:], ALU.mult, ALU.subtract)
            nc.scalar.activation(stdt[:], var[:], AF.Sqrt, bias=eps_sb[:])
            nc.vector.reciprocal(rt[:], stdt[:])
            mB = pp4.tile([128, NTOK], DT.float32, tag="mix")
            rB = pp4.tile([128, NTOK], DT.float32, tag="mix")
            nc.tensor.matmul(mB[:], lhsT=ones_sb[0:1, 0:128], rhs=tm[:], start=True, stop=True)
            nc.tensor.matmul(rB[:], lhsT=ones_sb[0:1, 0:128], rhs=rt[:], start=True, stop=True)
            for k in range(KT):
                t1 = app.tile([128, NTOK], DT.float32, tag="ap1")
                nc.vector.tensor_tensor(t1[:], x_sb[:, k, :], mB[:], ALU.subtract)
                if ln_affine:
                    nc.vector.tensor_tensor(t1[:], t1[:], rB[:], ALU.mult)
                    nc.scalar.activation(out_tile[:, k, :], t1[:], AF.Identity,
                                         bias=lnbt[:, k:k + 1], scale=lnst[:, k:k + 1])
                else:
                    nc.vector.tensor_tensor(out_tile[:, k, :], t1[:], rB[:], ALU.mult)

        # =================== transformer layers ===================
        with tc.tile_pool(name="pp", bufs=2, space="PSUM") as pp, \
                tc.tile_pool(name="pp4", bufs=4, space="PSUM") as pp4:
            for l in range(n_layers):
                with nc.named_scope(f"layer{l}"):
                    # ---- LN1: x -> h ----
                    emit_ln(pp, pp4, ln1s[l, :], ln1b[l, :], h_sb)

                    # ---- K^T, Q^T: [hs-pair partitions, tokens] ----
                    wk_t = [wp.tile([128, D], f32r, tag="wt", name=f"wk_t{l}_{k}") for k in range(KT)]
                    for k in range(KT):
                        nc.sync.dma_start(wk_t[k][:], wkt[l, k * 128:(k + 1) * 128, :].bitcast(f32r))
                    for h2 in range(KT):
                        pk = pp.tile([128, NTOK], DT.float32, tag="big")
                        for k in range(KT):
                            nc.tensor.matmul(pk[:], lhsT=wk_t[k][:, h2 * 128:(h2 + 1) * 128],
                                             rhs=h_sb[:, k, :], start=(k == 0), stop=(k == KT - 1))
                        nc.vector.tensor_copy(kT_sb[:, h2, :], pk[:])
                    wq_t = [wp.tile([128, D], f32r, tag="wt", name=f"wq_t{l}_{k}") for k in range(KT)]
                    for k in range(KT):
                        nc.sync.dma_start(wq_t[k][:], wqt[l, k * 128:(k + 1) * 128, :].bitcast(f32r))
                    for h2 in range(KT):
                        pq = pp.tile([128, NTOK], DT.float32, tag="big")
                        for k in range(KT):
                            nc.tensor.matmul(pq[:], lhsT=wq_t[k][:, h2 * 128:(h2 + 1) * 128],
                                             rhs=h_sb[:, k, :], start=(k == 0), stop=(k == KT - 1))
                        nc.vector.tensor_copy(qT_sb[:, h2, :], pq[:])

                    # ---- V in [token, (h hs)] layout + ones column ----
                    wv_t = [wp.tile([128, D], f32r, tag="wt", name=f"wv_t{l}_{k}") for k in range(KT)]
                    for k in range(KT):
                        nc.sync.dma_start(wv_t[k][:], wvt[l, k * 128:(k + 1) * 128, :].bitcast(f32r))
                    for tch in range(KT):
                        pv = pp.tile([128, D], DT.float32, tag="big")
                        for k in range(KT):
                            nc.tensor.matmul(pv[:], lhsT=h_sb[:, k, tch * 128:(tch + 1) * 128],
                                             rhs=wv_t[k][:], start=(k == 0), stop=(k == KT - 1))
                        nc.vector.tensor_copy(v_sb[:, tch, :, HS], ones_sb[:, 0:H])
                        nc.vector.tensor_copy(v_sb[:, tch, :, 0:HS],
                                              pv[:].rearrange("p (h x) -> p h x", h=H))

                    # ---- attention per (batch-row, head) ----
                    for br in range(BL):
                        for hh in range(H):
                            h2, ho = hh // 2, (hh % 2) * 64
                            base = br * T
                            patt = pp.tile([128, 2 * T], DT.float32, tag="att")
                            for jc in range(2):
                                nc.tensor.matmul(
                                    patt[:, jc * T:(jc + 1) * T],
                                    lhsT=qT_sb[ho:ho + 64, h2, base + jc * 128: base + (jc + 1) * 128],
                                    rhs=kT_sb[ho:ho + 64, h2, base:base + T],
                                    start=(jc == 0), stop=(jc == 1))
                            pt = pp_sb.tile([128, 2, T], f32r, tag="P")
                            for jc in range(2):
                                nc.scalar.activation(pt[:, jc, :], patt[:, jc * T:(jc + 1) * T],
                                                     AF.Exp, scale=SCALE)
                                nc.vector.tensor_tensor(pt[:, jc, :], pt[:, jc, :],
                                                        mask_sb[:, jc, :], ALU.mult)
                            pav = pp4.tile([HS + 1, 2 * T], DT.float32, tag="mix")
                            for jc in range(2):
                                nc.tensor.matmul(pav[:, 0:T],
                                                 lhsT=v_sb[:, 2 * br + jc, hh, :],
                                                 rhs=pt[:, jc, :],
                                                 start=(jc == 0), stop=(jc == 1))
                            rs = tiny.tile([1, T], f32r, tag="rs")
                            nc.vector.reciprocal(rs[:], pav[HS:HS + 1, 0:T])
                            nc.tensor.matmul(pav[0:64, T:2 * T], lhsT=ones_sb[0:1, 0:64],
                                             rhs=rs[:], start=True, stop=True)
                            nc.vector.tensor_copy(at_sb[ho:ho + 64, h2, base:base + T],
                                                  pav[0:64, 0:T])
                            nc.vector.tensor_tensor(at_sb[ho:ho + 64, h2, base:base + T],
                                                    at_sb[ho:ho + 64, h2, base:base + T],
                                                    pav[0:64, T:2 * T], ALU.mult)

                    # ---- proj + residual ----
                    wp_t = [wp.tile([128, D], f32r, tag="wt", name=f"wp_t{l}_{k}") for k in range(KT)]
                    for k in range(KT):
                        nc.sync.dma_start(wp_t[k][:], wpj[l, k * 128:(k + 1) * 128, :].bitcast(f32r))
                    if "proj" in with_biases:
                        bpr = rows.tile([1, D], f32r, tag="bpr")
                        nc.sync.dma_start(bpr[:], bpj[l:l + 1, :].bitcast(f32r))
                    for m in range(KT):
                        pj = pp.tile([128, NTOK], DT.float32, tag="big")
                        for k in range(KT):
                            nc.tensor.matmul(pj[:], lhsT=wp_t[k][:, m * 128:(m + 1) * 128],
                                             rhs=at_sb[:, k, :], start=(k == 0),
                                             stop=(k == KT - 1 and "proj" not in with_biases))
                        if "proj" in with_biases:
                            nc.tensor.matmul(pj[:], lhsT=bpr[0:1, m * 128:(m + 1) * 128],
                                             rhs=ones_sb[0:1, :], start=False, stop=True)
                        nc.vector.tensor_tensor(x_sb[:, m, :], x_sb[:, m, :], pj[:], ALU.add)

                    # ---- LN2: x -> h ----
                    emit_ln(pp, pp4, ln2s[l, :], ln2b[l, :], h_sb)

                    # ---- FFN ----
                    if "fc1" in with_biases:
                        b1r = rows.tile([1, F1], f32r, tag="b1r")
                        nc.sync.dma_start(b1r[:], b1d[l:l + 1, :].bitcast(f32r))
                    if "fc2" in with_biases:
                        b2r = rows.tile([1, D], f32r, tag="b2r")
                        nc.sync.dma_start(b2r[:], b2d[l:l + 1, :].bitcast(f32r))

                    pf = [pp4.tile([128, NTOK], DT.float32, tag="mix", name=f"pf{l}_{m}") for m in range(KT)]
                    if "fc2" in with_biases:
                        for m in range(KT):
                            nc.tensor.matmul(pf[m][:], lhsT=b2r[0:1, m * 128:(m + 1) * 128],
                                             rhs=ones_sb[0:1, :], start=True, stop=False)
                    for zg in range(NZ // 4):
                        w1_t = [w1p.tile([128, 512], f32r, tag="wt1", name=f"w1_t{l}_{zg}_{k}") for k in range(KT)]
                        for k in range(KT):
                            nc.sync.dma_start(
                                w1_t[k][:], w1[l, k * 128:(k + 1) * 128,
                                               zg * 512:(zg + 1) * 512].bitcast(f32r))
                        for zi in range(4):
                            z = zg * 4 + zi
                            pz = pp.tile([128, NTOK], DT.float32, tag="big")
                            for k in range(KT):
                                nc.tensor.matmul(pz[:], lhsT=w1_t[k][:, zi * 128:(zi + 1) * 128],
                                                 rhs=h_sb[:, k, :], start=(k == 0),
                                                 stop=(k == KT - 1 and "fc1" not in with_biases))
                            if "fc1" in with_biases:
                                nc.tensor.matmul(pz[:], lhsT=b1r[0:1, z * 128:(z + 1) * 128],
                                                 rhs=ones_sb[0:1, :], start=False, stop=True)
                            zt = ztp.tile([128, NTOK], f32r, tag="zt")
                            nc.scalar.activation(zt[:], pz[:], AF.Relu)
                            w2_t = w2p.tile([128, D], f32r, tag="w2")
                            nc.sync.dma_start(w2_t[:],
                                              w2[l, z * 128:(z + 1) * 128, :].bitcast(f32r))
                            for m in range(KT):
                                nc.tensor.matmul(pf[m][:], lhsT=w2_t[:, m * 128:(m + 1) * 128],
                                                 rhs=zt[:],
                                                 start=(z == 0 and "fc2" not in with_biases),
                                                 stop=(z == NZ - 1))
                    for m in range(KT):
                        nc.vector.tensor_tensor(x_sb[:, m, :], x_sb[:, m, :], pf[m][:], ALU.add)

            # ---- final LN: x -> h ----
            with nc.named_scope("lnf"):
                emit_ln(pp, pp4, lnfs[0, :], lnfb[0, :], h_sb)

        # =================== LM head ===================
        with tc.tile_pool(name="lmps", bufs=4, space="PSUM") as lmps:
            with nc.named_scope("lmhead"):
                for vc in range(n_vc):
                    wl_t = [wlmp.tile([128, VC], f32r, tag="wlm", name=f"wl_t{vc}_{k}") for k in range(KT)]
                    for k in range(KT):
                        nc.sync.dma_start(
                            wl_t[k][:], wlm[k * 128:(k + 1) * 128,
                                            vc * VC:(vc + 1) * VC].bitcast(f32r))
                    if "lm" in with_biases:
                        blr = rows.tile([1, VC], f32r, tag="blr")
                        nc.sync.dma_start(blr[:], blm[0:1, vc * VC:(vc + 1) * VC].bitcast(f32r))
                    for tch in range(KT):
                        pl = lmps.tile([128, VC], DT.float32, tag="lm")
                        for k in range(KT):
                            for vs in range(VC // 512):
                                nc.tensor.matmul(
                                    pl[:, vs * 512:(vs + 1) * 512],
                                    lhsT=h_sb[:, k, tch * 128:(tch + 1) * 128],
                                    rhs=wl_t[k][:, vs * 512:(vs + 1) * 512],
                                    start=(k == 0),
                                    stop=(k == KT - 1 and "lm" not in with_biases))
                        if "lm" in with_biases:
                            for vs in range(VC // 512):
                                nc.tensor.matmul(
                                    pl[:, vs * 512:(vs + 1) * 512],
                                    lhsT=ones_sb[0:1, 0:128],
                                    rhs=blr[0:1, vs * 512:(vs + 1) * 512],
                                    start=False, stop=True)
                        lg = scrp.tile([128, VC], DT.float32, tag="lg")
                        nc.vector.tensor_copy(lg[:], pl[:])
                        nc.gpsimd.dma_start(
                            logits[tch * 128:(tch + 1) * 128, vc * VC:(vc + 1) * VC], lg[:])
                        scr = scrp.tile([128, VC], DT.float32, tag="scr")
                        nc.scalar.activation(scr[:], pl[:], AF.Exp,
                                             accum_out=sacc_sb[:, tch, vc:vc + 1])
                for tch in range(KT):
                    nc.vector.tensor_reduce(sbuf_s[:, tch:tch + 1], sacc_sb[:, tch, 0:n_vc],
                                            axis=AX.X, op=ALU.add)
                    nc.sync.dma_start(ssum[tch, :], sbuf_s[:, tch:tch + 1])

    nc.compile()
    return nc


def prep_inputs(idx, targets, tok_emb, pos_emb, ln1_s, ln1_b, wk, wq, wv,
                w_proj, b_proj, ln2_s, ln2_b, w1, b1, w2, b2, lnf_s, lnf_b,
                w_lm, b_lm):
    """Host-side marshalling: per-core input dicts + bias presence flags."""
    f = lambda a: np.ascontiguousarray(np.asarray(a), dtype=np.float32)
    idx = np.asarray(idx)
    x0 = f(tok_emb)[idx.reshape(-1)].reshape(B, T, D) + f(pos_emb)[None]

    wkt = f(np.transpose(np.asarray(wk), (0, 2, 1, 3)).reshape(L, D, D))
    wqt = f(np.transpose(np.asarray(wq), (0, 2, 1, 3)).reshape(L, D, D))
    wvt = f(np.transpose(np.asarray(wv), (0, 2, 1, 3)).reshape(L, D, D))

    wlm_pad = np.zeros((D, VPAD), np.float32)
    wlm_pad[:, :V] = f(w_lm)
    blm_pad = np.full((1, VPAD), NEG, np.float32)
    blm_pad[0, :V] = f(b_lm)

    maskt = np.triu(np.ones((T, T), np.float32))
    ones = np.ones((128, 512), np.float32)

    shared = dict(
        wkt=wkt, wqt=wqt, wvt=wvt,
        wproj=f(w_proj), w1=f(w1), w2=f(w2),
        bproj=f(b_proj), b1=f(b1), b2=f(b2),
        ln1s=f(ln1_s), ln1b=f(ln1_b), ln2s=f(ln2_s), ln2b=f(ln2_b),
        lnfs=f(lnf_s).reshape(1, D), lnfb=f(lnf_b).reshape(1, D),
        wlm=wlm_pad, blm=blm_pad, maskt=maskt, ones=ones,
    )
    in_maps = []
    for c in range(NCORES):
        m = dict(shared)
        m["x0t"] = np.ascontiguousarray(x0[c * BL:(c + 1) * BL].reshape(NTOK, D).T)
        in_maps.append(m)

    wb = set()
    if np.any(np.asarray(b_proj)):
        wb.add("proj")
    if np.any(np.asarray(b1)):
        wb.add("fc1")
    if np.any(np.asarray(b2)):
        wb.add("fc2")
    if np.any(np.asarray(b_lm)):
        wb.add("lm")
    ln_affine = bool(
        np.any(np.asarray(ln1_b)) or np.any(np.asarray(ln2_b)) or np.any(np.asarray(lnf_b))
        or not np.all(np.asarray(ln1_s) == 1) or not np.all(np.asarray(ln2_s) == 1)
        or not np.all(np.asarray(lnf_s) == 1))
    return in_maps, frozenset(wb), ln_affine


_PROGRAM_CACHE = {}


def _get_program(with_biases, ln_affine):
    key = (with_biases, ln_affine)
    if key not in _PROGRAM_CACHE:
        _PROGRAM_CACHE[key] = build_program(with_biases=with_biases, ln_affine=ln_affine)
    return _PROGRAM_CACHE[key]


def run_device(in_maps, with_biases, ln_affine, trace=False):
    nc = _get_program(with_biases, ln_affine)
    res = run_bass_kernel_spmd(nc, in_maps, list(range(NCORES)), trace=trace)
    return nc, res


def assemble(res, targets, with_biases):
    logits_full = np.concatenate(
        [np.asarray(res.results[c]["logits"])[None] for c in range(NCORES)], axis=0)
    logits_full = logits_full.reshape(B * T, VPAD)[:, :V]
    S = np.concatenate([np.asarray(res.results[c]["ssum"]).reshape(NTOK)
                        for c in range(NCORES)]).astype(np.float64)
    if "lm" not in with_biases:
        S = S - float(N_PAD)  # zero-padded vocab entries contribute exp(0)=1 each
    tflat = np.asarray(targets).reshape(-1)
    picked = logits_full[np.arange(B * T), tflat]
    lse = np.log(S)
    loss = np.float32(-(picked.astype(np.float64) - lse).mean())
    logits_out = np.ascontiguousarray(logits_full.reshape(B, T, V), dtype=np.float32)
    return logits_out, loss


def kernel(**inputs):
    in_maps, wb, ln_affine = prep_inputs(**inputs)
    _, res = run_device(in_maps, wb, ln_affine)
    return assemble(res, inputs["targets"], wb)


# revision 11
# speedup vs baseline: 1.1160x; 1.1160x over previous
"""Trainium2 Bass kernel for nn_BigramModel (8-layer dense transformer + LM head).

Self-contained: hardcodes all shapes from the problem spec.
Sharding: data-parallel over batch (16 rows -> 2 rows/core on 8 cores).
All matmuls run in float32r (TF32-like, full PE rate); activations are kept
in transposed [d, token] layout so weights stream in their native
[d_in, d_out] layout as the stationary operand.

kernel(**inputs) -> (logits [16,256,32000] f32, loss scalar f32)
"""
import sys
import types
import numpy as np
from contextlib import ExitStack

# NTFF profile hook shim (agent image's antenv lacks axon_hooks).
try:
    import antenv.axon_hooks  # noqa: F401
except ModuleNotFoundError:
    try:
        from trn_agent_boot.trn_boot import _ntff_profile_via_ctypes
        _m = types.ModuleType("antenv.axon_hooks")
        _hook = _ntff_profile_via_ctypes("/opt/axon/libaxon_pjrt.so")
        _m.get_axon_ntff_profile_hook = lambda: _hook
        sys.modules["antenv.axon_hooks"] = _m
    except Exception:
        pass

import concourse.bass as bass  # noqa: F401
import concourse.tile as tile
import concourse.bacc as bacc
import concourse.mybir as mybir
from concourse.bass_utils import run_bass_kernel_spmd

DT = mybir.dt
AF = mybir.ActivationFunctionType
ALU = mybir.AluOpType
AX = mybir.AxisListType

# model dims
V, T, D, H, L, HS = 32000, 256, 512, 8, 8, 64
B = 16
EPS = 1e-5
SCALE = float(D) ** -0.5
NCORES = 8
BL = B // NCORES            # batch rows per core (2)
NTOK = BL * T               # tokens per core (512)
KT = D // 128               # d-tiles (4)
VPAD = 32768                # padded vocab
VC = 1024                   # lm-head vocab chunk
NVC = VPAD // VC            # 32 chunks
F1 = 4 * D                  # ffn dim 2048
NEG = -1.0e30
N_PAD = VPAD - V            # 768 zero-padded vocab entries


def build_program(n_layers=L, n_vc=NVC, with_biases=frozenset(), ln_affine=True):
    """Emit the per-core SPMD program. with_biases: subset of
    {"proj","fc1","fc2","lm"} for which bias matmuls are emitted."""
    nc = bacc.Bacc("TRN2", target_bir_lowering=False, debug=False)

    def din(name, shape):
        return nc.dram_tensor(name, shape, DT.float32, kind="ExternalInput").ap()

    x0t = din("x0t", [D, NTOK])
    wkt = din("wkt", [L, D, D])
    wqt = din("wqt", [L, D, D])
    wvt = din("wvt", [L, D, D])
    wpj = din("wproj", [L, D, D])
    w1 = din("w1", [L, D, F1])
    w2 = din("w2", [L, F1, D])
    bpj = din("bproj", [L, D])
    b1d = din("b1", [L, F1])
    b2d = din("b2", [L, D])
    ln1s = din("ln1s", [L, D])
    ln1b = din("ln1b", [L, D])
    ln2s = din("ln2s", [L, D])
    ln2b = din("ln2b", [L, D])
    lnfs = din("lnfs", [1, D])
    lnfb = din("lnfb", [1, D])
    wlm = din("wlm", [D, VPAD])
    blm = din("blm", [1, VPAD])
    maskt = din("maskt", [T, T])
    onesd = din("ones", [128, 512])

    logits = nc.dram_tensor("logits", [NTOK, VPAD], DT.float32, kind="ExternalOutput").ap()
    ssum = nc.dram_tensor("ssum", [KT, 128], DT.float32, kind="ExternalOutput").ap()

    f32r = DT.float32r
    NZ = F1 // 128  # 16

    with tile.TileContext(nc) as tc, ExitStack() as ctx, \
            nc.allow_low_precision(reason="float32r matmul pipeline by design"):
        st = ctx.enter_context(tc.tile_pool(name="state", bufs=1))
        wp = ctx.enter_context(tc.tile_pool(name="wp", bufs=6))      # [128,512] f32r, tag wt
        w1p = ctx.enter_context(tc.tile_pool(name="w1p", bufs=6))    # [128,512] f32r, tag wt1
        w2p = ctx.enter_context(tc.tile_pool(name="w2p", bufs=4))    # [128,512] f32r, tag w2
        ztp = ctx.enter_context(tc.tile_pool(name="ztp", bufs=3))
        sqp = ctx.enter_context(tc.tile_pool(name="sqp", bufs=2))
        app = ctx.enter_context(tc.tile_pool(name="app", bufs=3))
        pp_sb = ctx.enter_context(tc.tile_pool(name="ppsb", bufs=3))
        tiny = ctx.enter_context(tc.tile_pool(name="tiny", bufs=3))
        rows = ctx.enter_context(tc.tile_pool(name="rows", bufs=2))
        wlmp = ctx.enter_context(tc.tile_pool(name="wlmp", bufs=6))  # [128,1024] f32r
        scrp = ctx.enter_context(tc.tile_pool(name="scrp", bufs=2))  # [128,1024] f32

        # ---- persistent state tiles ----
        x_sb = st.tile([128, KT, NTOK], f32r, tag="x")
        h_sb = st.tile([128, KT, NTOK], f32r, tag="h")
        kT_sb = st.tile([128, KT, NTOK], f32r, tag="kT")
        qT_sb = st.tile([128, KT, NTOK], f32r, tag="qT")
        v_sb = st.tile([128, KT, H, HS + 1], f32r, tag="v")
        at_sb = st.tile([128, KT, NTOK], f32r, tag="at")
        ones_sb = st.tile([128, 512], f32r, tag="ones")
        mask_sb = st.tile([128, 2, T], f32r, tag="mask")
        sacc_sb = st.tile([128, KT, n_vc], DT.float32, tag="sacc")
        sbuf_s = st.tile([128, KT], DT.float32, tag="sfin")
        eps_sb = st.tile([1, 1], DT.float32, tag="eps")
        nc.vector.memset(eps_sb[:], EPS)

        nc.sync.dma_start(ones_sb[:], onesd.bitcast(f32r))
        for j in range(2):
            nc.sync.dma_start(mask_sb[:, j, :], maskt[j * 128:(j + 1) * 128, :].bitcast(f32r))
        for k in range(KT):
            nc.sync.dma_start(x_sb[:, k, :], x0t[k * 128:(k + 1) * 128, :].bitcast(f32r))

        def emit_ln(pp, pp4, s_dram, b_dram, out_tile):
            """LayerNorm over d (partitions): x_sb -> out_tile."""
            if ln_affine:
                lnst = rows.tile([128, KT], DT.float32, tag="lnst")
                lnbt = rows.tile([128, KT], DT.float32, tag="lnbt")
                nc.sync.dma_start(lnst[:], s_dram.rearrange("(c p) -> p c", p=128))
                nc.sync.dma_start(lnbt[:], b_dram.rearrange("(c p) -> p c", p=128))

            s1 = pp4.tile([1, NTOK], DT.float32, tag="mix")
            s2 = pp4.tile([1, NTOK], DT.float32, tag="mix")
            for k in range(KT):
                sq = sqp.tile([128, NTOK], f32r, tag="sq")
                xk = x_sb[:, k, :]
                nc.vector.tensor_tensor(sq[:], xk, xk, ALU.mult)
                nc.tensor.matmul(s1[:], lhsT=ones_sb[:, 0:1], rhs=xk,
                                 start=(k == 0), stop=(k == KT - 1))
                nc.tensor.matmul(s2[:], lhsT=ones_sb[:, 0:1], rhs=sq[:],
                                 start=(k == 0), stop=(k == KT - 1))
            tm = tiny.tile([1, NTOK], f32r, tag="tm")
            tm2 = tiny.tile([1, NTOK], DT.float32, tag="lnt")
            var = tiny.tile([1, NTOK], DT.float32, tag="lnt")
            stdt = tiny.tile([1, NTOK], DT.float32, tag="lnt")
            rt = tiny.tile([1, NTOK], f32r, tag="rt")
            nc.vector.tensor_scalar_mul(tm[:], s1[:], 1.0 / D)
            nc.vector.tensor_tensor(tm2[:], tm[:], tm[:], ALU.mult)
            nc.vector.scalar_tensor_tensor(var[:], s2[:], 1.0 / D, tm2[:], ALU.mult, ALU.subtract)
            nc.scalar.activation(stdt[:], var[:], AF.Sqrt, bias=eps_sb[:])
            rf = tiny.tile([1, NTOK], DT.float32, tag="rf")
            nc.vector.reciprocal_approx_fast(rf[:], stdt[:])
            nc.vector.tensor_copy(rt[:], rf[:])
            mB = pp4.tile([128, NTOK], DT.float32, tag="mix")
            rB = pp4.tile([128, NTOK], DT.float32, tag="mix")
            nc.tensor.matmul(mB[:], lhsT=ones_sb[0:1, 0:128], rhs=tm[:], start=True, stop=True)
            nc.tensor.matmul(rB[:], lhsT=ones_sb[0:1, 0:128], rhs=rt[:], start=True, stop=True)
            for k in range(KT):
                t1 = app.tile([128, NTOK], DT.float32, tag="ap1")
                nc.vector.tensor_tensor(t1[:], x_sb[:, k, :], mB[:], ALU.subtract)
                if ln_affine:
                    nc.vector.tensor_tensor(t1[:], t1[:], rB[:], ALU.mult)
                    nc.scalar.activation(out_tile[:, k, :], t1[:], AF.Identity,
                                         bias=lnbt[:, k:k + 1], scale=lnst[:, k:k + 1])
                else:
                    nc.vector.tensor_tensor(out_tile[:, k, :], t1[:], rB[:], ALU.mult)

        # =================== transformer layers ===================
        with tc.tile_pool(name="pp", bufs=2, space="PSUM") as pp, \
                tc.tile_pool(name="pp4", bufs=4, space="PSUM") as pp4:
            for l in range(n_layers):
                with nc.named_scope(f"layer{l}"):
                    # ---- LN1: x -> h ----
                    emit_ln(pp, pp4, ln1s[l, :], ln1b[l, :], h_sb)

                    # ---- K^T, Q^T: [hs-pair partitions, tokens] ----
                    wk_t = [wp.tile([128, D], f32r, tag="wt", name=f"wk_t{l}_{k}") for k in range(KT)]
                    for k in range(KT):
                        nc.sync.dma_start(wk_t[k][:], wkt[l, k * 128:(k + 1) * 128, :].bitcast(f32r))
                    for h2 in range(KT):
                        pk = pp.tile([128, NTOK], DT.float32, tag="big")
                        for k in range(KT):
                            nc.tensor.matmul(pk[:], lhsT=wk_t[k][:, h2 * 128:(h2 + 1) * 128],
                                             rhs=h_sb[:, k, :], start=(k == 0), stop=(k == KT - 1))
                        nc.vector.tensor_copy(kT_sb[:, h2, :], pk[:])
                    wq_t = [wp.tile([128, D], f32r, tag="wt", name=f"wq_t{l}_{k}") for k in range(KT)]
                    for k in range(KT):
                        nc.sync.dma_start(wq_t[k][:], wqt[l, k * 128:(k + 1) * 128, :].bitcast(f32r))
                    for h2 in range(KT):
                        pq = pp.tile([128, NTOK], DT.float32, tag="big")
                        for k in range(KT):
                            nc.tensor.matmul(pq[:], lhsT=wq_t[k][:, h2 * 128:(h2 + 1) * 128],
                                             rhs=h_sb[:, k, :], start=(k == 0), stop=(k == KT - 1))
                        nc.vector.tensor_copy(qT_sb[:, h2, :], pq[:])

                    # ---- V in [token, (h hs)] layout + ones column ----
                    wv_t = [wp.tile([128, D], f32r, tag="wt", name=f"wv_t{l}_{k}") for k in range(KT)]
                    for k in range(KT):
                        nc.sync.dma_start(wv_t[k][:], wvt[l, k * 128:(k + 1) * 128, :].bitcast(f32r))
                    for tch in range(KT):
                        pv = pp.tile([128, D], DT.float32, tag="big")
                        for k in range(KT):
                            nc.tensor.matmul(pv[:], lhsT=h_sb[:, k, tch * 128:(tch + 1) * 128],
                                             rhs=wv_t[k][:], start=(k == 0), stop=(k == KT - 1))
                        nc.vector.tensor_copy(v_sb[:, tch, :, HS], ones_sb[:, 0:H])
                        nc.vector.tensor_copy(v_sb[:, tch, :, 0:HS],
                                              pv[:].rearrange("p (h x) -> p h x", h=H))

                    # ---- attention per (batch-row, head) ----
                    for br in range(BL):
                        for hh in range(H):
                            h2, ho = hh // 2, (hh % 2) * 64
                            base = br * T
                            patt = pp.tile([128, 2 * T], DT.float32, tag="att")
                            for jc in range(2):
                                nc.tensor.matmul(
                                    patt[:, jc * T:(jc + 1) * T],
                                    lhsT=qT_sb[ho:ho + 64, h2, base + jc * 128: base + (jc + 1) * 128],
                                    rhs=kT_sb[ho:ho + 64, h2, base:base + T],
                                    start=(jc == 0), stop=(jc == 1))
                            pt = pp_sb.tile([128, 2, T], f32r, tag="P")
                            for jc in range(2):
                                nc.scalar.activation(pt[:, jc, :], patt[:, jc * T:(jc + 1) * T],
                                                     AF.Exp, scale=SCALE)
                                nc.vector.tensor_tensor(pt[:, jc, :], pt[:, jc, :],
                                                        mask_sb[:, jc, :], ALU.mult)
                            pav = pp4.tile([HS + 1, 2 * T], DT.float32, tag="mix")
                            for jc in range(2):
                                nc.tensor.matmul(pav[:, 0:T],
                                                 lhsT=v_sb[:, 2 * br + jc, hh, :],
                                                 rhs=pt[:, jc, :],
                                                 start=(jc == 0), stop=(jc == 1))
                            rs = tiny.tile([1, T], f32r, tag="rs")
                            sden = tiny.tile([1, T], DT.float32, tag="sden")
                            nc.vector.tensor_copy(sden[:], pav[HS:HS + 1, 0:T])
                            rsf = tiny.tile([1, T], DT.float32, tag="rsf")
                            nc.vector.reciprocal_approx_fast(rsf[:], sden[:])
                            nc.vector.tensor_copy(rs[:], rsf[:])
                            nc.tensor.matmul(pav[0:64, T:2 * T], lhsT=ones_sb[0:1, 0:64],
                                             rhs=rs[:], start=True, stop=True)
                            nc.vector.tensor_copy(at_sb[ho:ho + 64, h2, base:base + T],
                                                  pav[0:64, 0:T])
                            nc.vector.tensor_tensor(at_sb[ho:ho + 64, h2, base:base + T],
                                                    at_sb[ho:ho + 64, h2, base:base + T],
                                                    pav[0:64, T:2 * T], ALU.mult)

                    # ---- proj + residual ----
                    wp_t = [wp.tile([128, D], f32r, tag="wt", name=f"wp_t{l}_{k}") for k in range(KT)]
                    for k in range(KT):
                        nc.sync.dma_start(wp_t[k][:], wpj[l, k * 128:(k + 1) * 128, :].bitcast(f32r))
                    if "proj" in with_biases:
                        bpr = rows.tile([1, D], f32r, tag="bpr")
                        nc.sync.dma_start(bpr[:], bpj[l:l + 1, :].bitcast(f32r))
                    for m in range(KT):
                        pj = pp.tile([128, NTOK], DT.float32, tag="big")
                        for k in range(KT):
                            nc.tensor.matmul(pj[:], lhsT=wp_t[k][:, m * 128:(m + 1) * 128],
                                             rhs=at_sb[:, k, :], start=(k == 0),
                                             stop=(k == KT - 1 and "proj" not in with_biases))
                        if "proj" in with_biases:
                            nc.tensor.matmul(pj[:], lhsT=bpr[0:1, m * 128:(m + 1) * 128],
                                             rhs=ones_sb[0:1, :], start=False, stop=True)
                        nc.vector.tensor_tensor(x_sb[:, m, :], x_sb[:, m, :], pj[:], ALU.add)

                    # ---- LN2: x -> h ----
                    emit_ln(pp, pp4, ln2s[l, :], ln2b[l, :], h_sb)

                    # ---- FFN ----
                    if "fc1" in with_biases:
                        b1r = rows.tile([1, F1], f32r, tag="b1r")
                        nc.sync.dma_start(b1r[:], b1d[l:l + 1, :].bitcast(f32r))
                    if "fc2" in with_biases:
                        b2r = rows.tile([1, D], f32r, tag="b2r")
                        nc.sync.dma_start(b2r[:], b2d[l:l + 1, :].bitcast(f32r))

                    pf = [pp4.tile([128, NTOK], DT.float32, tag="mix", name=f"pf{l}_{m}") for m in range(KT)]
                    if "fc2" in with_biases:
                        for m in range(KT):
                            nc.tensor.matmul(pf[m][:], lhsT=b2r[0:1, m * 128:(m + 1) * 128],
                                             rhs=ones_sb[0:1, :], start=True, stop=False)
                    for zg in range(NZ // 4):
                        w1_t = [w1p.tile([128, 512], f32r, tag="wt1", name=f"w1_t{l}_{zg}_{k}") for k in range(KT)]
                        for k in range(KT):
                            nc.sync.dma_start(
                                w1_t[k][:], w1[l, k * 128:(k + 1) * 128,
                                               zg * 512:(zg + 1) * 512].bitcast(f32r))
                        for zi in range(4):
                            z = zg * 4 + zi
                            pz = pp.tile([128, NTOK], DT.float32, tag="big")
                            for k in range(KT):
                                nc.tensor.matmul(pz[:], lhsT=w1_t[k][:, zi * 128:(zi + 1) * 128],
                                                 rhs=h_sb[:, k, :], start=(k == 0),
                                                 stop=(k == KT - 1 and "fc1" not in with_biases))
                            if "fc1" in with_biases:
                                nc.tensor.matmul(pz[:], lhsT=b1r[0:1, z * 128:(z + 1) * 128],
                                                 rhs=ones_sb[0:1, :], start=False, stop=True)
                            zt = ztp.tile([128, NTOK], f32r, tag="zt")
                            nc.scalar.activation(zt[:], pz[:], AF.Relu)
                            w2_t = w2p.tile([128, D], f32r, tag="w2")
                            nc.sync.dma_start(w2_t[:],
                                              w2[l, z * 128:(z + 1) * 128, :].bitcast(f32r))
                            for m in range(KT):
                                nc.tensor.matmul(pf[m][:], lhsT=w2_t[:, m * 128:(m + 1) * 128],
                                                 rhs=zt[:],
                                                 start=(z == 0 and "fc2" not in with_biases),
                                                 stop=(z == NZ - 1))
                    for m in range(KT):
                        nc.vector.tensor_tensor(x_sb[:, m, :], x_sb[:, m, :], pf[m][:], ALU.add)

            # ---- final LN: x -> h ----
            with nc.named_scope("lnf"):
                emit_ln(pp, pp4, lnfs[0, :], lnfb[0, :], h_sb)

        # =================== LM head ===================
        with tc.tile_pool(name="lmps", bufs=4, space="PSUM") as lmps:
            with nc.named_scope("lmhead"):
                for vc in range(n_vc):
                    wl_t = [wlmp.tile([128, VC], f32r, tag="wlm", name=f"wl_t{vc}_{k}") for k in range(KT)]
                    for k in range(KT):
                        nc.sync.dma_start(
                            wl_t[k][:], wlm[k * 128:(k + 1) * 128,
                                            vc * VC:(vc + 1) * VC].bitcast(f32r))
                    if "lm" in with_biases:
                        blr = rows.tile([1, VC], f32r, tag="blr")
                        nc.sync.dma_start(blr[:], blm[0:1, vc * VC:(vc + 1) * VC].bitcast(f32r))
                    for tch in range(KT):
                        pl = lmps.tile([128, VC], DT.float32, tag="lm")
                        for k in range(KT):
                            for vs in range(VC // 512):
                                nc.tensor.matmul(
                                    pl[:, vs * 512:(vs + 1) * 512],
                                    lhsT=h_sb[:, k, tch * 128:(tch + 1) * 128],
                                    rhs=wl_t[k][:, vs * 512:(vs + 1) * 512],
                                    start=(k == 0),
                                    stop=(k == KT - 1 and "lm" not in with_biases))
                        if "lm" in with_biases:
                            for vs in range(VC // 512):
                                nc.tensor.matmul(
                                    pl[:, vs * 512:(vs + 1) * 512],
                                    lhsT=ones_sb[0:1, 0:128],
                                    rhs=blr[0:1, vs * 512:(vs + 1) * 512],
                                    start=False, stop=True)
                        lg = scrp.tile([128, VC], DT.float32, tag="lg")
                        nc.vector.tensor_copy(lg[:], pl[:])
                        nc.gpsimd.dma_start(
                            logits[tch * 128:(tch + 1) * 128, vc * VC:(vc + 1) * VC], lg[:])
                        scr = scrp.tile([128, VC], DT.float32, tag="scr")
                        nc.scalar.activation(scr[:], pl[:], AF.Exp,
                                             accum_out=sacc_sb[:, tch, vc:vc + 1])
                for tch in range(KT):
                    nc.vector.tensor_reduce(sbuf_s[:, tch:tch + 1], sacc_sb[:, tch, 0:n_vc],
                                            axis=AX.X, op=ALU.add)
                    nc.sync.dma_start(ssum[tch, :], sbuf_s[:, tch:tch + 1])

    nc.compile()
    return nc


def prep_inputs(idx, targets, tok_emb, pos_emb, ln1_s, ln1_b, wk, wq, wv,
                w_proj, b_proj, ln2_s, ln2_b, w1, b1, w2, b2, lnf_s, lnf_b,
                w_lm, b_lm):
    """Host-side marshalling: per-core input dicts + bias presence flags."""
    f = lambda a: np.ascontiguousarray(np.asarray(a), dtype=np.float32)
    idx = np.asarray(idx)
    x0 = f(tok_emb)[idx.reshape(-1)].reshape(B, T, D) + f(pos_emb)[None]

    wkt = f(np.transpose(np.asarray(wk), (0, 2, 1, 3)).reshape(L, D, D))
    wqt = f(np.transpose(np.asarray(wq), (0, 2, 1, 3)).reshape(L, D, D))
    wvt = f(np.transpose(np.asarray(wv), (0, 2, 1, 3)).reshape(L, D, D))

    wlm_pad = np.zeros((D, VPAD), np.float32)
    wlm_pad[:, :V] = f(w_lm)
    blm_pad = np.full((1, VPAD), NEG, np.float32)
    blm_pad[0, :V] = f(b_lm)

    maskt = np.triu(np.ones((T, T), np.float32))
    ones = np.ones((128, 512), np.float32)

    shared = dict(
        wkt=wkt, wqt=wqt, wvt=wvt,
        wproj=f(w_proj), w1=f(w1), w2=f(w2),
        bproj=f(b_proj), b1=f(b1), b2=f(b2),
        ln1s=f(ln1_s), ln1b=f(ln1_b), ln2s=f(ln2_s), ln2b=f(ln2_b),
        lnfs=f(lnf_s).reshape(1, D), lnfb=f(lnf_b).reshape(1, D),
        wlm=wlm_pad, blm=blm_pad, maskt=maskt, ones=ones,
    )
    in_maps = []
    for c in range(NCORES):
        m = dict(shared)
        m["x0t"] = np.ascontiguousarray(x0[c * BL:(c + 1) * BL].reshape(NTOK, D).T)
        in_maps.append(m)

    wb = set()
    if np.any(np.asarray(b_proj)):
        wb.add("proj")
    if np.any(np.asarray(b1)):
        wb.add("fc1")
    if np.any(np.asarray(b2)):
        wb.add("fc2")
    if np.any(np.asarray(b_lm)):
        wb.add("lm")
    ln_affine = bool(
        np.any(np.asarray(ln1_b)) or np.any(np.asarray(ln2_b)) or np.any(np.asarray(lnf_b))
        or not np.all(np.asarray(ln1_s) == 1) or not np.all(np.asarray(ln2_s) == 1)
        or not np.all(np.asarray(lnf_s) == 1))
    return in_maps, frozenset(wb), ln_affine


_PROGRAM_CACHE = {}


def _get_program(with_biases, ln_affine):
    key = (with_biases, ln_affine)
    if key not in _PROGRAM_CACHE:
        _PROGRAM_CACHE[key] = build_program(with_biases=with_biases, ln_affine=ln_affine)
    return _PROGRAM_CACHE[key]


def run_device(in_maps, with_biases, ln_affine, trace=False):
    nc = _get_program(with_biases, ln_affine)
    res = run_bass_kernel_spmd(nc, in_maps, list(range(NCORES)), trace=trace)
    return nc, res


def assemble(res, targets, with_biases):
    logits_full = np.concatenate(
        [np.asarray(res.results[c]["logits"])[None] for c in range(NCORES)], axis=0)
    logits_full = logits_full.reshape(B * T, VPAD)[:, :V]
    S = np.concatenate([np.asarray(res.results[c]["ssum"]).reshape(NTOK)
                        for c in range(NCORES)]).astype(np.float64)
    if "lm" not in with_biases:
        S = S - float(N_PAD)  # zero-padded vocab entries contribute exp(0)=1 each
    tflat = np.asarray(targets).reshape(-1)
    picked = logits_full[np.arange(B * T), tflat]
    lse = np.log(S)
    loss = np.float32(-(picked.astype(np.float64) - lse).mean())
    logits_out = np.ascontiguousarray(logits_full.reshape(B, T, V), dtype=np.float32)
    return logits_out, loss


def kernel(**inputs):
    in_maps, wb, ln_affine = prep_inputs(**inputs)
    _, res = run_device(in_maps, wb, ln_affine)
    return assemble(res, inputs["targets"], wb)


# revision 12
# speedup vs baseline: 1.2416x; 1.1126x over previous
"""Trainium2 Bass kernel for nn_BigramModel (8-layer dense transformer + LM head).

Self-contained: hardcodes all shapes from the problem spec.
Sharding: data-parallel over batch (16 rows -> 2 rows/core on 8 cores).
All matmuls run in float32r (TF32-like, full PE rate); activations are kept
in transposed [d, token] layout so weights stream in their native
[d_in, d_out] layout as the stationary operand.

kernel(**inputs) -> (logits [16,256,32000] f32, loss scalar f32)
"""
import sys
import types
import numpy as np
from contextlib import ExitStack

# NTFF profile hook shim (agent image's antenv lacks axon_hooks).
try:
    import antenv.axon_hooks  # noqa: F401
except ModuleNotFoundError:
    try:
        from trn_agent_boot.trn_boot import _ntff_profile_via_ctypes
        _m = types.ModuleType("antenv.axon_hooks")
        _hook = _ntff_profile_via_ctypes("/opt/axon/libaxon_pjrt.so")
        _m.get_axon_ntff_profile_hook = lambda: _hook
        sys.modules["antenv.axon_hooks"] = _m
    except Exception:
        pass

import concourse.bass as bass  # noqa: F401
import concourse.tile as tile
import concourse.bacc as bacc
import concourse.mybir as mybir
from concourse.bass_utils import run_bass_kernel_spmd

DT = mybir.dt
AF = mybir.ActivationFunctionType
ALU = mybir.AluOpType
AX = mybir.AxisListType

# model dims
V, T, D, H, L, HS = 32000, 256, 512, 8, 8, 64
B = 16
EPS = 1e-5
SCALE = float(D) ** -0.5
NCORES = 8
BL = B // NCORES            # batch rows per core (2)
NTOK = BL * T               # tokens per core (512)
KT = D // 128               # d-tiles (4)
VPAD = 32768                # padded vocab
VC = 1024                   # lm-head vocab chunk
NVC = VPAD // VC            # 32 chunks
F1 = 4 * D                  # ffn dim 2048
NEG = -1.0e30
N_PAD = VPAD - V            # 768 zero-padded vocab entries


def build_program(n_layers=L, n_vc=NVC, with_biases=frozenset(), ln_affine=True):
    """Emit the per-core SPMD program. with_biases: subset of
    {"proj","fc1","fc2","lm"} for which bias matmuls are emitted."""
    nc = bacc.Bacc("TRN2", target_bir_lowering=False, debug=False)

    def din(name, shape):
        return nc.dram_tensor(name, shape, DT.float32, kind="ExternalInput").ap()

    x0t = din("x0t", [D, NTOK])
    wkt = din("wkt", [L, D, D])
    wqt = din("wqt", [L, D, D])
    wvt = din("wvt", [L, D, D])
    wpj = din("wproj", [L, D, D])
    w1 = din("w1", [L, D, F1])
    w2 = din("w2", [L, F1, D])
    bpj = din("bproj", [L, D])
    b1d = din("b1", [L, F1])
    b2d = din("b2", [L, D])
    ln1s = din("ln1s", [L, D])
    ln1b = din("ln1b", [L, D])
    ln2s = din("ln2s", [L, D])
    ln2b = din("ln2b", [L, D])
    lnfs = din("lnfs", [1, D])
    lnfb = din("lnfb", [1, D])
    wlm = din("wlm", [D, VPAD])
    blm = din("blm", [1, VPAD])
    maskt = din("maskt", [T, T])
    onesd = din("ones", [128, 512])

    logits = nc.dram_tensor("logits", [NTOK, VPAD], DT.float32, kind="ExternalOutput").ap()
    ssum = nc.dram_tensor("ssum", [KT, 128], DT.float32, kind="ExternalOutput").ap()

    f32r = DT.float32r
    NZ = F1 // 128  # 16

    with tile.TileContext(nc) as tc, ExitStack() as ctx, \
            nc.allow_low_precision(reason="float32r matmul pipeline by design"):
        st = ctx.enter_context(tc.tile_pool(name="state", bufs=1))
        wp = ctx.enter_context(tc.tile_pool(name="wp", bufs=6))      # [128,512] f32r, tag wt
        w1p = ctx.enter_context(tc.tile_pool(name="w1p", bufs=6))    # [128,512] f32r, tag wt1
        w2p = ctx.enter_context(tc.tile_pool(name="w2p", bufs=4))    # [128,512] f32r, tag w2
        ztp = ctx.enter_context(tc.tile_pool(name="ztp", bufs=3))
        sqp = ctx.enter_context(tc.tile_pool(name="sqp", bufs=2))
        app = ctx.enter_context(tc.tile_pool(name="app", bufs=3))
        pp_sb = ctx.enter_context(tc.tile_pool(name="ppsb", bufs=4))
        tiny = ctx.enter_context(tc.tile_pool(name="tiny", bufs=4))
        rows = ctx.enter_context(tc.tile_pool(name="rows", bufs=2))
        wlmp = ctx.enter_context(tc.tile_pool(name="wlmp", bufs=6))  # [128,1024] f32r
        scrp = ctx.enter_context(tc.tile_pool(name="scrp", bufs=3))  # [128,1024] f32

        # ---- persistent state tiles ----
        x_sb = st.tile([128, KT, NTOK], f32r, tag="x")
        h_sb = st.tile([128, KT, NTOK], f32r, tag="h")
        kT_sb = st.tile([128, KT, NTOK], f32r, tag="kT")
        qT_sb = st.tile([128, KT, NTOK], f32r, tag="qT")
        v_sb = st.tile([128, KT, H, HS + 1], f32r, tag="v")
        at_sb = st.tile([128, KT, NTOK], f32r, tag="at")
        ones_sb = st.tile([128, 512], f32r, tag="ones")
        mask_sb = st.tile([128, 2, T], f32r, tag="mask")
        sacc_sb = st.tile([128, KT, n_vc], DT.float32, tag="sacc")
        sbuf_s = st.tile([128, KT], DT.float32, tag="sfin")
        eps_sb = st.tile([1, 1], DT.float32, tag="eps")
        nc.vector.memset(eps_sb[:], EPS)

        nc.sync.dma_start(ones_sb[:], onesd.bitcast(f32r))
        for j in range(2):
            nc.sync.dma_start(mask_sb[:, j, :], maskt[j * 128:(j + 1) * 128, :].bitcast(f32r))
        for k in range(KT):
            nc.sync.dma_start(x_sb[:, k, :], x0t[k * 128:(k + 1) * 128, :].bitcast(f32r))

        def emit_ln(pp, pp4, s_dram, b_dram, out_tile):
            """LayerNorm over d (partitions): x_sb -> out_tile."""
            if ln_affine:
                lnst = rows.tile([128, KT], DT.float32, tag="lnst")
                lnbt = rows.tile([128, KT], DT.float32, tag="lnbt")
                nc.sync.dma_start(lnst[:], s_dram.rearrange("(c p) -> p c", p=128))
                nc.sync.dma_start(lnbt[:], b_dram.rearrange("(c p) -> p c", p=128))

            s1 = pp4.tile([1, NTOK], DT.float32, tag="mix")
            s2 = pp4.tile([1, NTOK], DT.float32, tag="mix")
            for k in range(KT):
                sq = sqp.tile([128, NTOK], f32r, tag="sq")
                xk = x_sb[:, k, :]
                nc.vector.tensor_tensor(sq[:], xk, xk, ALU.mult)
                nc.tensor.matmul(s1[:], lhsT=ones_sb[:, 0:1], rhs=xk,
                                 start=(k == 0), stop=(k == KT - 1))
                nc.tensor.matmul(s2[:], lhsT=ones_sb[:, 0:1], rhs=sq[:],
                                 start=(k == 0), stop=(k == KT - 1))
            tm = tiny.tile([1, NTOK], f32r, tag="tm")
            tm2 = tiny.tile([1, NTOK], DT.float32, tag="lnt")
            var = tiny.tile([1, NTOK], DT.float32, tag="lnt")
            stdt = tiny.tile([1, NTOK], DT.float32, tag="lnt")
            rt = tiny.tile([1, NTOK], f32r, tag="rt")
            nc.vector.tensor_scalar_mul(tm[:], s1[:], 1.0 / D)
            nc.vector.tensor_tensor(tm2[:], tm[:], tm[:], ALU.mult)
            nc.vector.scalar_tensor_tensor(var[:], s2[:], 1.0 / D, tm2[:], ALU.mult, ALU.subtract)
            nc.scalar.activation(stdt[:], var[:], AF.Sqrt, bias=eps_sb[:])
            rf = tiny.tile([1, NTOK], DT.float32, tag="rf")
            nc.vector.reciprocal_approx_fast(rf[:], stdt[:])
            nc.vector.tensor_copy(rt[:], rf[:])
            mB = pp4.tile([128, NTOK], DT.float32, tag="mix")
            rB = pp4.tile([128, NTOK], DT.float32, tag="mix")
            nc.tensor.matmul(mB[:], lhsT=ones_sb[0:1, 0:128], rhs=tm[:], start=True, stop=True)
            nc.tensor.matmul(rB[:], lhsT=ones_sb[0:1, 0:128], rhs=rt[:], start=True, stop=True)
            for k in range(KT):
                t1 = app.tile([128, NTOK], DT.float32, tag="ap1")
                nc.vector.tensor_tensor(t1[:], x_sb[:, k, :], mB[:], ALU.subtract)
                if ln_affine:
                    nc.vector.tensor_tensor(t1[:], t1[:], rB[:], ALU.mult)
                    nc.scalar.activation(out_tile[:, k, :], t1[:], AF.Identity,
                                         bias=lnbt[:, k:k + 1], scale=lnst[:, k:k + 1])
                else:
                    nc.vector.tensor_tensor(out_tile[:, k, :], t1[:], rB[:], ALU.mult)

        # =================== transformer layers ===================
        with tc.tile_pool(name="pp", bufs=2, space="PSUM") as pp, \
                tc.tile_pool(name="pp4", bufs=4, space="PSUM") as pp4:
            for l in range(n_layers):
                with nc.named_scope(f"layer{l}"):
                    # ---- LN1: x -> h ----
                    emit_ln(pp, pp4, ln1s[l, :], ln1b[l, :], h_sb)

                    # ---- K^T, Q^T: [hs-pair partitions, tokens] ----
                    wk_t = [wp.tile([128, D], f32r, tag="wt", name=f"wk_t{l}_{k}") for k in range(KT)]
                    for k in range(KT):
                        nc.sync.dma_start(wk_t[k][:], wkt[l, k * 128:(k + 1) * 128, :].bitcast(f32r))
                    for h2 in range(KT):
                        pk = pp.tile([128, NTOK], DT.float32, tag="big")
                        for k in range(KT):
                            nc.tensor.matmul(pk[:], lhsT=wk_t[k][:, h2 * 128:(h2 + 1) * 128],
                                             rhs=h_sb[:, k, :], start=(k == 0), stop=(k == KT - 1))
                        nc.vector.tensor_copy(kT_sb[:, h2, :], pk[:])
                    wq_t = [wp.tile([128, D], f32r, tag="wt", name=f"wq_t{l}_{k}") for k in range(KT)]
                    for k in range(KT):
                        nc.sync.dma_start(wq_t[k][:], wqt[l, k * 128:(k + 1) * 128, :].bitcast(f32r))
                    for h2 in range(KT):
                        pq = pp.tile([128, NTOK], DT.float32, tag="big")
                        for k in range(KT):
                            nc.tensor.matmul(pq[:], lhsT=wq_t[k][:, h2 * 128:(h2 + 1) * 128],
                                             rhs=h_sb[:, k, :], start=(k == 0), stop=(k == KT - 1))
                        nc.vector.tensor_copy(qT_sb[:, h2, :], pq[:])

                    # ---- V in [token, (h hs)] layout + ones column ----
                    wv_t = [wp.tile([128, D], f32r, tag="wt", name=f"wv_t{l}_{k}") for k in range(KT)]
                    for k in range(KT):
                        nc.sync.dma_start(wv_t[k][:], wvt[l, k * 128:(k + 1) * 128, :].bitcast(f32r))
                    for tch in range(KT):
                        pv = pp.tile([128, D], DT.float32, tag="big")
                        for k in range(KT):
                            nc.tensor.matmul(pv[:], lhsT=h_sb[:, k, tch * 128:(tch + 1) * 128],
                                             rhs=wv_t[k][:], start=(k == 0), stop=(k == KT - 1))
                        nc.vector.tensor_copy(v_sb[:, tch, :, HS], ones_sb[:, 0:H])
                        nc.vector.tensor_copy(v_sb[:, tch, :, 0:HS],
                                              pv[:].rearrange("p (h x) -> p h x", h=H))

                    # ---- attention per (batch-row, head) ----
                    for br in range(BL):
                        for hh in range(H):
                            h2, ho = hh // 2, (hh % 2) * 64
                            base = br * T
                            patt = pp.tile([128, 2 * T], DT.float32, tag="att")
                            for jc in range(2):
                                nc.tensor.matmul(
                                    patt[:, jc * T:(jc + 1) * T],
                                    lhsT=qT_sb[ho:ho + 64, h2, base + jc * 128: base + (jc + 1) * 128],
                                    rhs=kT_sb[ho:ho + 64, h2, base:base + T],
                                    start=(jc == 0), stop=(jc == 1))
                            pt = pp_sb.tile([128, 2, T], f32r, tag="P")
                            for jc in range(2):
                                nc.scalar.activation(pt[:, jc, :], patt[:, jc * T:(jc + 1) * T],
                                                     AF.Exp, scale=SCALE)
                                nc.gpsimd.tensor_tensor(pt[:, jc, :], pt[:, jc, :],
                                                        mask_sb[:, jc, :], ALU.mult)
                            pav = pp4.tile([HS + 1, 2 * T], DT.float32, tag="mix")
                            for jc in range(2):
                                nc.tensor.matmul(pav[:, 0:T],
                                                 lhsT=v_sb[:, 2 * br + jc, hh, :],
                                                 rhs=pt[:, jc, :],
                                                 start=(jc == 0), stop=(jc == 1))
                            rs = tiny.tile([1, T], f32r, tag="rs")
                            sden = tiny.tile([1, T], DT.float32, tag="sden")
                            nc.vector.tensor_copy(sden[:], pav[HS:HS + 1, 0:T])
                            rsf = tiny.tile([1, T], DT.float32, tag="rsf")
                            nc.vector.reciprocal_approx_fast(rsf[:], sden[:])
                            nc.vector.tensor_copy(rs[:], rsf[:])
                            nc.tensor.matmul(pav[0:64, T:2 * T], lhsT=ones_sb[0:1, 0:64],
                                             rhs=rs[:], start=True, stop=True)
                            nc.scalar.activation(at_sb[ho:ho + 64, h2, base:base + T],
                                                 pav[0:64, 0:T], AF.Copy)
                            nc.vector.tensor_tensor(at_sb[ho:ho + 64, h2, base:base + T],
                                                    at_sb[ho:ho + 64, h2, base:base + T],
                                                    pav[0:64, T:2 * T], ALU.mult)

                    # ---- proj + residual ----
                    wp_t = [wp.tile([128, D], f32r, tag="wt", name=f"wp_t{l}_{k}") for k in range(KT)]
                    for k in range(KT):
                        nc.sync.dma_start(wp_t[k][:], wpj[l, k * 128:(k + 1) * 128, :].bitcast(f32r))
                    if "proj" in with_biases:
                        bpr = rows.tile([1, D], f32r, tag="bpr")
                        nc.sync.dma_start(bpr[:], bpj[l:l + 1, :].bitcast(f32r))
                    for m in range(KT):
                        pj = pp.tile([128, NTOK], DT.float32, tag="big")
                        for k in range(KT):
                            nc.tensor.matmul(pj[:], lhsT=wp_t[k][:, m * 128:(m + 1) * 128],
                                             rhs=at_sb[:, k, :], start=(k == 0),
                                             stop=(k == KT - 1 and "proj" not in with_biases))
                        if "proj" in with_biases:
                            nc.tensor.matmul(pj[:], lhsT=bpr[0:1, m * 128:(m + 1) * 128],
                                             rhs=ones_sb[0:1, :], start=False, stop=True)
                        nc.vector.tensor_tensor(x_sb[:, m, :], x_sb[:, m, :], pj[:], ALU.add)

                    # ---- LN2: x -> h ----
                    emit_ln(pp, pp4, ln2s[l, :], ln2b[l, :], h_sb)

                    # ---- FFN ----
                    if "fc1" in with_biases:
                        b1r = rows.tile([1, F1], f32r, tag="b1r")
                        nc.sync.dma_start(b1r[:], b1d[l:l + 1, :].bitcast(f32r))
                    if "fc2" in with_biases:
                        b2r = rows.tile([1, D], f32r, tag="b2r")
                        nc.sync.dma_start(b2r[:], b2d[l:l + 1, :].bitcast(f32r))

                    pf = [pp4.tile([128, NTOK], DT.float32, tag="mix", name=f"pf{l}_{m}") for m in range(KT)]
                    if "fc2" in with_biases:
                        for m in range(KT):
                            nc.tensor.matmul(pf[m][:], lhsT=b2r[0:1, m * 128:(m + 1) * 128],
                                             rhs=ones_sb[0:1, :], start=True, stop=False)
                    for zg in range(NZ // 4):
                        w1_t = [w1p.tile([128, 512], f32r, tag="wt1", name=f"w1_t{l}_{zg}_{k}") for k in range(KT)]
                        for k in range(KT):
                            nc.sync.dma_start(
                                w1_t[k][:], w1[l, k * 128:(k + 1) * 128,
                                               zg * 512:(zg + 1) * 512].bitcast(f32r))
                        for zi in range(4):
                            z = zg * 4 + zi
                            pz = pp.tile([128, NTOK], DT.float32, tag="big")
                            for k in range(KT):
                                nc.tensor.matmul(pz[:], lhsT=w1_t[k][:, zi * 128:(zi + 1) * 128],
                                                 rhs=h_sb[:, k, :], start=(k == 0),
                                                 stop=(k == KT - 1 and "fc1" not in with_biases))
                            if "fc1" in with_biases:
                                nc.tensor.matmul(pz[:], lhsT=b1r[0:1, z * 128:(z + 1) * 128],
                                                 rhs=ones_sb[0:1, :], start=False, stop=True)
                            zt = ztp.tile([128, NTOK], f32r, tag="zt")
                            nc.scalar.activation(zt[:], pz[:], AF.Relu)
                            w2_t = w2p.tile([128, D], f32r, tag="w2")
                            nc.sync.dma_start(w2_t[:],
                                              w2[l, z * 128:(z + 1) * 128, :].bitcast(f32r))
                            for m in range(KT):
                                nc.tensor.matmul(pf[m][:], lhsT=w2_t[:, m * 128:(m + 1) * 128],
                                                 rhs=zt[:],
                                                 start=(z == 0 and "fc2" not in with_biases),
                                                 stop=(z == NZ - 1))
                    for m in range(KT):
                        nc.vector.tensor_tensor(x_sb[:, m, :], x_sb[:, m, :], pf[m][:], ALU.add)

            # ---- final LN: x -> h ----
            with nc.named_scope("lnf"):
                emit_ln(pp, pp4, lnfs[0, :], lnfb[0, :], h_sb)

        # =================== LM head ===================
        with tc.tile_pool(name="lmps", bufs=4, space="PSUM") as lmps:
            with nc.named_scope("lmhead"):
                for vc in range(n_vc):
                    wl_t = [wlmp.tile([128, VC], f32r, tag="wlm", name=f"wl_t{vc}_{k}") for k in range(KT)]
                    for k in range(KT):
                        nc.sync.dma_start(
                            wl_t[k][:], wlm[k * 128:(k + 1) * 128,
                                            vc * VC:(vc + 1) * VC].bitcast(f32r))
                    if "lm" in with_biases:
                        blr = rows.tile([1, VC], f32r, tag="blr")
                        nc.sync.dma_start(blr[:], blm[0:1, vc * VC:(vc + 1) * VC].bitcast(f32r))
                    for tch in range(KT):
                        pl = lmps.tile([128, VC], DT.float32, tag="lm")
                        for k in range(KT):
                            for vs in range(VC // 512):
                                nc.tensor.matmul(
                                    pl[:, vs * 512:(vs + 1) * 512],
                                    lhsT=h_sb[:, k, tch * 128:(tch + 1) * 128],
                                    rhs=wl_t[k][:, vs * 512:(vs + 1) * 512],
                                    start=(k == 0),
                                    stop=(k == KT - 1 and "lm" not in with_biases))
                        if "lm" in with_biases:
                            for vs in range(VC // 512):
                                nc.tensor.matmul(
                                    pl[:, vs * 512:(vs + 1) * 512],
                                    lhsT=ones_sb[0:1, 0:128],
                                    rhs=blr[0:1, vs * 512:(vs + 1) * 512],
                                    start=False, stop=True)
                        lg = scrp.tile([128, VC], DT.float32, tag="lg")
                        nc.vector.tensor_copy(lg[:], pl[:])
                        nc.gpsimd.dma_start(
                            logits[tch * 128:(tch + 1) * 128, vc * VC:(vc + 1) * VC], lg[:])
                        scr = scrp.tile([128, VC], DT.float32, tag="scr")
                        nc.scalar.activation(scr[:], pl[:], AF.Exp,
                                             accum_out=sacc_sb[:, tch, vc:vc + 1])
                for tch in range(KT):
                    nc.vector.tensor_reduce(sbuf_s[:, tch:tch + 1], sacc_sb[:, tch, 0:n_vc],
                                            axis=AX.X, op=ALU.add)
                    nc.sync.dma_start(ssum[tch, :], sbuf_s[:, tch:tch + 1])

    nc.compile()
    return nc


def prep_inputs(idx, targets, tok_emb, pos_emb, ln1_s, ln1_b, wk, wq, wv,
                w_proj, b_proj, ln2_s, ln2_b, w1, b1, w2, b2, lnf_s, lnf_b,
                w_lm, b_lm):
    """Host-side marshalling: per-core input dicts + bias presence flags."""
    f = lambda a: np.ascontiguousarray(np.asarray(a), dtype=np.float32)
    idx = np.asarray(idx)
    x0 = f(tok_emb)[idx.reshape(-1)].reshape(B, T, D) + f(pos_emb)[None]

    wkt = f(np.transpose(np.asarray(wk), (0, 2, 1, 3)).reshape(L, D, D))
    wqt = f(np.transpose(np.asarray(wq), (0, 2, 1, 3)).reshape(L, D, D))
    wvt = f(np.transpose(np.asarray(wv), (0, 2, 1, 3)).reshape(L, D, D))

    wlm_pad = np.zeros((D, VPAD), np.float32)
    wlm_pad[:, :V] = f(w_lm)
    blm_pad = np.full((1, VPAD), NEG, np.float32)
    blm_pad[0, :V] = f(b_lm)

    maskt = np.triu(np.ones((T, T), np.float32))
    ones = np.ones((128, 512), np.float32)

    shared = dict(
        wkt=wkt, wqt=wqt, wvt=wvt,
        wproj=f(w_proj), w1=f(w1), w2=f(w2),
        bproj=f(b_proj), b1=f(b1), b2=f(b2),
        ln1s=f(ln1_s), ln1b=f(ln1_b), ln2s=f(ln2_s), ln2b=f(ln2_b),
        lnfs=f(lnf_s).reshape(1, D), lnfb=f(lnf_b).reshape(1, D),
        wlm=wlm_pad, blm=blm_pad, maskt=maskt, ones=ones,
    )
    in_maps = []
    for c in range(NCORES):
        m = dict(shared)
        m["x0t"] = np.ascontiguousarray(x0[c * BL:(c + 1) * BL].reshape(NTOK, D).T)
        in_maps.append(m)

    wb = set()
    if np.any(np.asarray(b_proj)):
        wb.add("proj")
    if np.any(np.asarray(b1)):
        wb.add("fc1")
    if np.any(np.asarray(b2)):
        wb.add("fc2")
    if np.any(np.asarray(b_lm)):
        wb.add("lm")
    ln_affine = bool(
        np.any(np.asarray(ln1_b)) or np.any(np.asarray(ln2_b)) or np.any(np.asarray(lnf_b))
        or not np.all(np.asarray(ln1_s) == 1) or not np.all(np.asarray(ln2_s) == 1)
        or not np.all(np.asarray(lnf_s) == 1))
    return in_maps, frozenset(wb), ln_affine


_PROGRAM_CACHE = {}


def _get_program(with_biases, ln_affine):
    key = (with_biases, ln_affine)
    if key not in _PROGRAM_CACHE:
        _PROGRAM_CACHE[key] = build_program(with_biases=with_biases, ln_affine=ln_affine)
    return _PROGRAM_CACHE[key]


def run_device(in_maps, with_biases, ln_affine, trace=False):
    nc = _get_program(with_biases, ln_affine)
    res = run_bass_kernel_spmd(nc, in_maps, list(range(NCORES)), trace=trace)
    return nc, res


def assemble(res, targets, with_biases):
    logits_full = np.concatenate(
        [np.asarray(res.results[c]["logits"])[None] for c in range(NCORES)], axis=0)
    logits_full = logits_full.reshape(B * T, VPAD)[:, :V]
    S = np.concatenate([np.asarray(res.results[c]["ssum"]).reshape(NTOK)
                        for c in range(NCORES)]).astype(np.float64)
    if "lm" not in with_biases:
        S = S - float(N_PAD)  # zero-padded vocab entries contribute exp(0)=1 each
    tflat = np.asarray(targets).reshape(-1)
    picked = logits_full[np.arange(B * T), tflat]
    lse = np.log(S)
    loss = np.float32(-(picked.astype(np.float64) - lse).mean())
    logits_out = np.ascontiguousarray(logits_full.reshape(B, T, V), dtype=np.float32)
    return logits_out, loss


def kernel(**inputs):
    in_maps, wb, ln_affine = prep_inputs(**inputs)
    _, res = run_device(in_maps, wb, ln_affine)
    return assemble(res, inputs["targets"], wb)
